# revision 135
# baseline (speedup 1.0000x reference)
"""Trainium2 Bass kernel for nn_Attention_29326036697657 (sparse_attention).

Dual-input attention with SE (channel) / SA (spatial) gates.
Sharding: data-parallel over batch B=64 across 8 cores (8 batches/core).

Algebraic simplifications vs the reference (same as baseline):
  - qxo/qyo/attnx are dead code in the reference -> comp 0 of Wqkv unused.
  - vy = vx (reference quirk) -> only one V, from x's qkv.
  - dots(qx,kx)+dots(qx2,kx) = dots(qx*(1+g), kx)   (SE channel gate)
  - dots(qy,ky)+dots(qy2,ky) = dots(qy*(1+s), ky)   (SA spatial gate scales
    q rows by query position)
  - xo = z @ Wp^2 + (b@Wp + b), computed host-side as wp2/bp2.
Softmax without max-subtraction (logits are O(1)).

v4 design (cost-model driven; 445us -> 224.9us, 1.98x over baseline):
  - All activations/weights bf16 on device (1 cyc/row matmuls); outputs
    written bf16 and widened to f32 on the host.
  - Inputs arrive HOST-pre-transposed as bf16 slabs [128, 6*1152]
    (chunk-major), so no on-device input transposes at all; weights
    host-packed to [128, 6*768] slabs, one DMA each (DMA instruction
    count matters: each holds the shared HWDGE device ~625ns).
  - SE channel gate: stats reduced per qx chunk as it is produced; the
    (1+g) gate is FUSED into the kx psum evictions (dots(q(1+g),k) ==
    dots(q,(1+g)k) for a per-(batch,channel) gate), so no separate
    scaling pass exists on the critical path.
  - SA spatial gate: channel sum/max via gpsimd partition_all_reduce;
    padded grids live in single partitions and are embedded by compute
    engines (no DMA); the 5x5 conv runs as 5 dy-shifted matmuls over an
    x-pre-shifted [10=(dx,ch), b*16*12] operand built with 10 small
    gpsimd DMAs; gate broadcast via gpsimd partition_broadcast.
  - Attention: S/exp/av per (b,head) with the softmax denominator
    produced by a ones-column in the V tiles; z transposed via 6 PE
    transposes into one [128,432] bf16 psum tile + a single DVE
    eviction per (b,i); per-batch transposes deferred one batch.
  - One continuous pipeline: ky and proj units (which only need the zT
    token rows their t-tile covers) are dripped into the attention
    loops after each half-batch, keeping PE busy while ACT grinds the
    192 softmax exps (the second-busiest engine).
  - Bias rows broadcast once via gpsimd partition_broadcast and fused
    into the psum->stage evictions on DVE.
  - ACT runs only {Copy, Exp}: relu is a DVE max0 and every sigmoid is
    computed as 1/(1+exp(-x)) on the Exp table, so the two 1.28us
    mid-kernel ACT table reloads disappear (one load at t~0.7us).
"""

import sys

sys.path.insert(0, "/opt/trn_rl_repo")

from contextlib import ExitStack

import numpy as np

import concourse.bass as bass
import concourse.bacc as bacc_mod
import concourse.bass_isa as bass_isa
import concourse.mybir as mybir
import concourse.tile as tile
from concourse.masks import make_identity

# ---------------------------------------------------------------- constants
DIM = 768
HEADS = 12
PATCH = 12
N = PATCH * PATCH          # 144
B = 64
RED = 16
HID = DIM // RED           # 48
HD = DIM // HEADS          # 64
SCALE = HD ** -0.5         # 0.125

NCORES = 8
BC = B // NCORES           # 8 batches per core
NT = BC * N                # 1152 tokens per core
CH = DIM // 128            # 6 channel chunks
NROW = NT // 128           # 9 row chunks
NF = 384                   # matmul moving-dim chunk
NNF = NT // NF             # 3
MC = 72                    # m/n chunk within one batch (144 = 2*72)

F32 = mybir.dt.float32
F32R = mybir.dt.float32r
BF16 = mybir.dt.bfloat16
AX = mybir.AxisListType
AF = mybir.ActivationFunctionType
ALU = mybir.AluOpType
RO = bass_isa.ReduceOp

_COMPILED = {}


def build_program():
    nc = bacc_mod.Bacc()

    # ---- DRAM I/O (all layouts are host-prepared) ----
    xT_d = nc.dram_tensor("xT", [128, CH * NT], BF16, kind="ExternalInput")
    yT_d = nc.dram_tensor("yT", [128, CH * NT], BF16, kind="ExternalInput")
    wq_d = nc.dram_tensor("wq", [128, CH * DIM], BF16, kind="ExternalInput")
    wk_d = nc.dram_tensor("wk", [128, CH * DIM], BF16, kind="ExternalInput")
    wv_d = nc.dram_tensor("wv", [128, CH * DIM], BF16, kind="ExternalInput")
    wp_d = nc.dram_tensor("wp", [128, CH * DIM], BF16, kind="ExternalInput")
    wp2_d = nc.dram_tensor("wp2", [128, CH * DIM], BF16, kind="ExternalInput")
    bp_d = nc.dram_tensor("bp", [1, DIM], BF16, kind="ExternalInput")
    bp2_d = nc.dram_tensor("bp2", [1, DIM], BF16, kind="ExternalInput")
    sw1m_d = nc.dram_tensor("sw1m", [128, CH * HID], BF16, kind="ExternalInput")
    sw1x_d = nc.dram_tensor("sw1x", [128, CH * HID], BF16, kind="ExternalInput")
    sw2_d = nc.dram_tensor("sw2", [HID, DIM], BF16, kind="ExternalInput")
    cw_d = nc.dram_tensor("cw", [10, 5], BF16, kind="ExternalInput")
    cb_d = nc.dram_tensor("cb", [1, 1], F32, kind="ExternalInput")
    outs_d = {
        nm: nc.dram_tensor(nm, [NT, DIM], BF16, kind="ExternalOutput")
        for nm in ("x1", "y1", "xo", "yo")
    }

    with tile.TileContext(nc) as tc:
        _body(nc, tc, xT_d, yT_d, wq_d, wk_d, wv_d, wp_d, wp2_d, bp_d, bp2_d,
              sw1m_d, sw1x_d, sw2_d, cw_d, cb_d, outs_d)
    nc.compile()
    return nc


def _body(nc, tc, xT_d, yT_d, wq_d, wk_d, wv_d, wp_d, wp2_d, bp_d, bp2_d,
          sw1m_d, sw1x_d, sw2_d, cw_d, cb_d, outs_d):
    est = ExitStack()
    with est:
        # ---------------- const / small tiles ----------------
        const = est.enter_context(tc.tile_pool(name="const", bufs=1))
        ident_bf = const.tile([128, 128], BF16, tag="idbf", name="idbf")
        make_identity(nc, ident_bf)
        cb_sb = const.tile([1, 1], F32, tag="cb", name="cb")
        cw_sb = const.tile([10, 5], BF16, tag="cw", name="cw")
        # touch every activation function once while ACT is idle so Bacc's
        # table-load instructions land at t~0 instead of mid-kernel
        warm = const.tile([1, 1], F32, tag="warm", name="warm")
        nc.vector.memset(warm, 0.5)
        nc.scalar.activation(warm, warm, AF.Exp)

        # ---------------- pools (LIFO nesting per side) ----------------
        # left open order: vt, qky, wk, yT, qkx, sa, wqv, xT, se, vs;
        # closes: se(SE-b), vs/xT/wqv (v done), sa(SA-b), qkx/yT/wk (attn_x
        # done), then pw/stage open and everything lives to the end.
        vt_est = ExitStack()
        vt_pool = vt_est.enter_context(tc.tile_pool(name="vt", bufs=1))
        vt = [[vt_pool.tile([MC, HEADS * 65], BF16, tag=f"v{b}_{j}",
                            name=f"v{b}_{j}") for j in range(2)]
              for b in range(BC)]

        qky_est = ExitStack()
        qky_pool = qky_est.enter_context(tc.tile_pool(name="qky", bufs=1))
        qy = qky_pool.tile([128, CH * NT], BF16, tag="qy", name="qy")
        ky = qky_pool.tile([128, CH * NT], BF16, tag="ky", name="ky")
        wp_s = qky_pool.tile([128, CH * DIM], BF16, tag="wp", name="wp")
        wp2_s = qky_pool.tile([128, CH * DIM], BF16, tag="wp2", name="wp2")
        bias_bc = {}
        bstg = {}
        for bname in ("b1", "b2"):
            bias_bc[bname] = qky_pool.tile([128, DIM], BF16, tag=f"bc{bname}",
                                           name=f"bc{bname}")
            bstg[bname] = qky_pool.tile([1, DIM], BF16, tag=f"bs{bname}",
                                        name=f"bs{bname}")
        stage_pool = qky_pool

        wk_est = ExitStack()
        wk_pool = wk_est.enter_context(tc.tile_pool(name="wkp", bufs=1))
        wk_s = wk_pool.tile([128, CH * DIM], BF16, tag="wk", name="wk")

        yT_est = ExitStack()
        yT_pool = yT_est.enter_context(tc.tile_pool(name="yTp", bufs=1))
        yT_s = yT_pool.tile([128, CH * NT], BF16, tag="yT", name="yT")

        qkx_est = ExitStack()
        qkx_pool = qkx_est.enter_context(tc.tile_pool(name="qkx", bufs=1))
        qx = qkx_pool.tile([128, CH * NT], BF16, tag="qx", name="qx")
        kx = qkx_pool.tile([128, CH * NT], BF16, tag="kx", name="kx")

        sa_est = ExitStack()
        sa_pool = sa_est.enter_context(tc.tile_pool(name="sa", bufs=1))

        wqv_est = ExitStack()
        wqv_pool = wqv_est.enter_context(tc.tile_pool(name="wqv", bufs=1))
        wq_s = wqv_pool.tile([128, CH * DIM], BF16, tag="wq", name="wq")
        wv_s = wqv_pool.tile([128, CH * DIM], BF16, tag="wv", name="wv")

        xT_est = ExitStack()
        xT_pool = xT_est.enter_context(tc.tile_pool(name="xTp", bufs=1))
        xT_s = xT_pool.tile([128, CH * NT], BF16, tag="xT", name="xT")

        # startup DMA order: interleave wq/xT sixths so q matmuls start ASAP
        T6 = CH * DIM // 6
        TT = CH * NT // 6
        for i in range(6):
            nc.sync.dma_start(out=wq_s[:, i * T6:(i + 1) * T6],
                              in_=wq_d[:, i * T6:(i + 1) * T6])
            nc.sync.dma_start(out=xT_s[:, i * TT:(i + 1) * TT],
                              in_=xT_d[:, i * TT:(i + 1) * TT])
        HT = CH * NT // 2
        nc.sync.dma_start(out=yT_s[:, 0:HT], in_=yT_d[:, 0:HT])
        nc.sync.dma_start(out=yT_s[:, HT:], in_=yT_d[:, HT:])
        nc.sync.dma_start(out=wk_s, in_=wk_d[:, :])
        nc.sync.dma_start(out=wv_s, in_=wv_d[:, :])
        nc.sync.dma_start(out=cb_sb, in_=cb_d[:, :])
        nc.sync.dma_start(out=cw_sb, in_=cw_d[:, :])
        nc.sync.dma_start(out=wp_s, in_=wp_d[:, :])
        nc.sync.dma_start(out=wp2_s, in_=wp2_d[:, :])
        for bname, b_d in (("b1", bp_d), ("b2", bp2_d)):
            nc.sync.dma_start(out=bstg[bname], in_=b_d[:, :])
            nc.gpsimd.partition_broadcast(bias_bc[bname], bstg[bname], 128)

        evict_ctr = [0]

        def evict(dst, src):
            # alternate psum->sbuf eviction between DVE and ACT
            if evict_ctr[0] % 2 == 0:
                nc.vector.tensor_copy(dst, src)
            else:
                nc.scalar.copy(dst, src)
            evict_ctr[0] += 1

        # ---------------- phase 1: q projections ----------------
        qkv_est = ExitStack()
        qkv_ps = qkv_est.enter_context(
            tc.tile_pool(name="qkvps", bufs=5, space="PSUM"))

        def qproj(w_s, src_s, dst_s, m, pool=None, tag="qkv", eng=None):
            # one m-chunk of a [768->768] projection, transposed output
            for nf in range(NNF):
                ps = (pool or qkv_ps).tile([128, NF], F32, tag=tag, name=tag)
                for kc in range(CH):
                    nc.tensor.matmul(
                        ps,
                        w_s[:, kc * DIM + m * 128:kc * DIM + (m + 1) * 128],
                        src_s[:, kc * NT + nf * NF:kc * NT + (nf + 1) * NF],
                        start=(kc == 0), stop=(kc == CH - 1))
                dst = dst_s[:, m * NT + nf * NF:m * NT + (nf + 1) * NF]
                if eng is None:
                    evict(dst, ps)
                else:
                    eng.tensor_copy(dst, ps)

        # ------- phase 1+2: qx with SE stats interleaved, then SE fc -------
        se_est = ExitStack()
        se_pool = se_est.enter_context(tc.tile_pool(name="se", bufs=1))
        sw1m_s = se_pool.tile([128, CH * HID], BF16, tag="s1m", name="s1m")
        sw1x_s = se_pool.tile([128, CH * HID], BF16, tag="s1x", name="s1x")
        sw2_s = se_pool.tile([HID, DIM], BF16, tag="sw2", name="sw2")
        nc.sync.dma_start(out=sw1m_s, in_=sw1m_d[:, :])
        nc.sync.dma_start(out=sw1x_s, in_=sw1x_d[:, :])
        nc.sync.dma_start(out=sw2_s, in_=sw2_d[:, :])
        sums = [se_pool.tile([128, BC], BF16, tag=f"sum{c}", name=f"sum{c}")
                for c in range(CH)]
        maxs = [se_pool.tile([128, BC], BF16, tag=f"max{c}", name=f"max{c}")
                for c in range(CH)]
        # SE fc1 accumulates incrementally as each qx chunk's stats land,
        # so g1 (needed by the gate-fused kx evictions) is ready ~when the
        # last qx chunk finishes instead of a full fc-chain latency later.
        se_ps_est = ExitStack()
        se_ps = se_ps_est.enter_context(
            tc.tile_pool(name="seps", bufs=1, space="PSUM"))
        fc1b = se_ps.tile([HID, 2 * BC], F32, tag="fc1", name="fc1")
        fc1 = {0: fc1b[:, 0:BC], 1: fc1b[:, BC:2 * BC]}
        with nc.allow_low_precision(reason="SE gate stats tolerate bf16"):
            for m in range(CH):
                qproj(wq_s, xT_s, qx, m)
                q3 = qx[:, m * NT:(m + 1) * NT].rearrange("p (b n) -> p b n", n=N)
                nc.vector.reduce_sum(sums[m], q3, axis=AX.X)
                nc.vector.reduce_max(maxs[m], q3, axis=AX.X)
                nc.tensor.matmul(fc1[0], sw1m_s[:, m * HID:(m + 1) * HID],
                                 sums[m], start=(m == 0), stop=(m == CH - 1))
                nc.tensor.matmul(fc1[1], sw1x_s[:, m * HID:(m + 1) * HID],
                                 maxs[m], start=(m == 0), stop=(m == CH - 1))

        hidb = se_pool.tile([HID, 2 * BC], BF16, tag="hidb", name="hidb")
        with nc.allow_low_precision(reason="SE gate tolerates bf16"):
            nc.vector.tensor_scalar(hidb, fc1b, 0.0, 0.0, ALU.max, ALU.bypass)
        g1s = []
        lp = nc.allow_low_precision(reason="SE gate tolerates bf16")
        lp.__enter__()
        for c in range(CH):
            ps2 = se_ps.tile([128, 2 * BC], F32, tag="fc2", name="fc2")
            nc.tensor.matmul(ps2[:, 0:BC], sw2_s[:, c * 128:(c + 1) * 128],
                             hidb[:, 0:BC], start=True, stop=True)
            nc.tensor.matmul(ps2[:, BC:2 * BC], sw2_s[:, c * 128:(c + 1) * 128],
                             hidb[:, BC:2 * BC], start=True, stop=True)
            # sigmoid = 1/(1+exp(-x)) using the Exp table (keeps ACT on one
            # activation set -> no mid-kernel table reloads)
            eneg = se_pool.tile([128, 2 * BC], F32, tag="en", name="en", bufs=2)
            nc.scalar.activation(eneg, ps2, AF.Exp, scale=-1.0)
            nc.vector.tensor_scalar(eneg, eneg, 1.0, 0.0, ALU.add, ALU.bypass)
            sg = se_pool.tile([128, 2 * BC], BF16, tag=f"sg{c}", name=f"sg{c}")
            nc.vector.reciprocal(sg, eneg)
            g1 = se_pool.tile([128, BC], BF16, tag=f"g1{c}", name=f"g1{c}")
            nc.vector.tensor_add(g1, sg[:, 0:BC], sg[:, BC:2 * BC])
            g1s.append(g1)
        lp.__exit__(None, None, None)
        se_ps_est.close()

        for m in range(CH):
            qproj(wq_s, yT_s, qy, m)

        # ---------------- phase 3: SA-a (spatial stats of qy) ----------------
        sa_ps_est = ExitStack()
        sa_ps = sa_ps_est.enter_context(
            tc.tile_pool(name="saps", bufs=1, space="PSUM"))
        accm = sa_pool.tile([128, NT], BF16, tag="accm", name="accm")
        accs = sa_pool.tile([128, NT], BF16, tag="accs", name="accs")
        nc.vector.tensor_max(accm, qy[:, 0:NT], qy[:, NT:2 * NT])
        with nc.allow_low_precision(reason="SA mean tolerates bf16"):
            nc.vector.tensor_add(accs, qy[:, 0:NT], qy[:, NT:2 * NT])
            for c in range(2, CH):
                nc.vector.tensor_max(accm, accm, qy[:, c * NT:(c + 1) * NT])
                nc.vector.tensor_add(accs, accs, qy[:, c * NT:(c + 1) * NT])
        pmax = sa_pool.tile([128, NT], BF16, tag="pmax", name="pmax")
        nc.gpsimd.partition_all_reduce(pmax, accm, 128, RO.max)
        pavg = sa_pool.tile([128, NT], BF16, tag="pavg", name="pavg")
        nc.gpsimd.partition_all_reduce(pavg, accs, 128, RO.add)
        # Padded 16x16 grids per channel, each in ONE partition so compute
        # engines can write them (no partition-base-1 access). Channel 0 =
        # mean (as SUM; /DIM folded into conv weight), channel 1 = max.
        mean_pad = sa_pool.tile([1, BC * 256], BF16, tag="mpad", name="mpad")
        max_pad = sa_pool.tile([1, BC * 256], BF16, tag="xpad", name="xpad")
        nc.vector.memset(mean_pad, 0.0)
        nc.vector.memset(max_pad, 0.0)
        mpadw = mean_pad.rearrange("p (b yy xx) -> p b yy xx", yy=16, xx=16)
        xpadw = max_pad.rearrange("p (b yy xx) -> p b yy xx", yy=16, xx=16)
        NG = 2 * N  # 288-col group = 2 batches
        for g in range(4):
            nc.scalar.copy(
                mpadw[0:1, 2 * g:2 * g + 2, 2:14, 2:14],
                pavg[0:1, g * NG:(g + 1) * NG].rearrange(
                    "p (b yy xx) -> p b yy xx", yy=12, xx=12))
            nc.scalar.copy(
                xpadw[0:1, 2 * g:2 * g + 2, 2:14, 2:14],
                pmax[0:1, g * NG:(g + 1) * NG].rearrange(
                    "p (b yy xx) -> p b yy xx", yy=12, xx=12))
        # x-pre-shifted conv operand: opx[(dx,ch), (b, py16, x12)] =
        # grid_ch[b, py, x+dx]; 10 small DMAs, then the 5x5 conv is 5
        # dy-shifted matmuls per 2-batch group contracting over (dx,ch).
        opx = sa_pool.tile([10, BC * 16 * PATCH], BF16, tag="opx", name="opx")
        opx4 = opx.rearrange("p (b yy xx) -> p b yy xx", yy=16, xx=PATCH)
        for dx in range(5):
            for chn, grid in ((0, mpadw), (1, xpadw)):
                nc.gpsimd.dma_start(out=opx4[2 * dx + chn:2 * dx + chn + 1],
                                    in_=grid[:, :, :, dx:dx + PATCH])

        # ---------------- phase 4: kx (SE gate fused into evictions) -------
        # eviction: kx = (1+g) * psum, with g broadcast along n per batch.
        # 288-col tiles (2 batches) so the broadcast view stays b-aligned.
        NK = 2 * N  # 288

        def kproj(m):
            for nf in range(4):
                ps = qkv_ps.tile([128, NK], F32, tag="qkv", name="qkv")
                for kc in range(CH):
                    nc.tensor.matmul(
                        ps,
                        wk_s[:, kc * DIM + m * 128:kc * DIM + (m + 1) * 128],
                        xT_s[:, kc * NT + nf * NK:kc * NT + (nf + 1) * NK],
                        start=(kc == 0), stop=(kc == CH - 1))
                dst3 = kx[:, m * NT + nf * NK:m * NT + (nf + 1) * NK].rearrange(
                    "p (b n) -> p b n", n=N)
                g3 = g1s[m][:, 2 * nf:2 * nf + 2].unsqueeze(2).to_broadcast(
                    (128, 2, N))
                nc.vector.scalar_tensor_tensor(
                    dst3, g3, 1.0, ps.rearrange("p (b n) -> p b n", n=N),
                    ALU.add, ALU.mult)

        for m in range(CH):
            kproj(m)

        # ---------------- phase 6: v (natural layout + resplit) ----------------
        vs_est = ExitStack()
        vs_pool = vs_est.enter_context(tc.tile_pool(name="vs", bufs=4))
        vstage = [None] * NROW
        for t in range(NROW):
            vstage[t] = vs_pool.tile([128, HEADS * 65], BF16, tag="vs",
                                     name="vs")
            ones_ap = vstage[t].rearrange("p (h o) -> p h o", o=65)[:, :, 64:65]
            nc.vector.memset(ones_ap, 1.0)
            for half in range(2):
                ps = qkv_ps.tile([128, NF], F32, tag="qkv", name="qkv")
                for kc in range(CH):
                    nc.tensor.matmul(
                        ps, xT_s[:, kc * NT + t * 128:kc * NT + (t + 1) * 128],
                        wv_s[:, kc * DIM + half * NF:kc * DIM + (half + 1) * NF],
                        start=(kc == 0), stop=(kc == CH - 1))
                dst3 = vstage[t].rearrange("p (h o) -> p h o", o=65)[
                    :, half * 6:(half + 1) * 6, 0:64]
                evict(dst3, ps.rearrange("p (h d) -> p h d", d=64))
            # re-split each (b,j) slab to its vt tile as soon as covered
            for b in range(BC):
                for j in range(2):
                    row0 = b * N + j * MC
                    if (row0 + MC - 1) // 128 != t:
                        continue
                    pos = 0
                    while pos < MC:
                        tt = (row0 + pos) // 128
                        r0 = (row0 + pos) % 128
                        cnt = min(128 - r0, MC - pos)
                        nc.sync.dma_start(out=vt[b][j][pos:pos + cnt, :],
                                          in_=vstage[tt][r0:r0 + cnt, :])
                        pos += cnt

        vs_est.close()
        se_est.close()
        xT_est.close()
        wqv_est.close()

        # ---------------- phase 7: SA-b (conv gate, scale qy) ----------------
        # 1+sigmoid(x+cb) computed as 1 + 1/(1+exp(-x-cb)); the ACT op uses
        # the Exp table (cb arrives host-negated in cb_sb)
        t_row1 = sa_pool.tile([1, NT], BF16, tag="trow1", name="trow1")
        for g in range(4):
            ps = sa_ps.tile([1, NG], F32, tag="sam", name="sam")
            for dy in range(5):
                v = opx4[:, 2 * g:2 * g + 2, dy:dy + PATCH, :]
                nc.tensor.matmul(ps, cw_sb[:, dy:dy + 1],
                                 v.rearrange("p b yy xx -> p b (yy xx)"),
                                 start=(dy == 0), stop=(dy == 4))
            erow = sa_pool.tile([1, NG], F32, tag="er", name="er", bufs=2)
            nc.scalar.activation(erow, ps, AF.Exp, scale=-1.0, bias=cb_sb)
            nc.vector.tensor_scalar(erow, erow, 1.0, 0.0, ALU.add, ALU.bypass)
            rrow = sa_pool.tile([1, NG], F32, tag="rr", name="rr", bufs=2)
            nc.vector.reciprocal(rrow, erow)
            with nc.allow_low_precision(reason="SA gate tolerates bf16"):
                nc.vector.tensor_scalar(t_row1[:, g * NG:(g + 1) * NG], rrow,
                                        1.0, 0.0, ALU.add, ALU.bypass)
        t_bc = sa_pool.tile([128, NT], BF16, tag="tbc", name="tbc")
        nc.gpsimd.partition_broadcast(t_bc, t_row1, 128)
        with tc.high_priority(offset=-1500):
            for c in range(CH):
                qslice = qy[:, c * NT:(c + 1) * NT]
                nc.vector.tensor_tensor(qslice, qslice, t_bc, op=ALU.mult)
        sa_ps_est.close()
        qkv_est.close()

        # ---------------- attention ----------------
        # right-side stack: zTy under zTx (zTx closes first, after proj_x)
        zTy_est = ExitStack()
        zTy_pool = zTy_est.enter_context(
            tc.tile_pool(name="zTy", bufs=1, side="right"))
        zTy = zTy_pool.tile([128, CH * NT], BF16, tag="zTy", name="zTy")
        zTx_est = ExitStack()
        zTx_pool = zTx_est.enter_context(
            tc.tile_pool(name="zTx", bufs=1, side="right"))
        zTx = zTx_pool.tile([128, CH * NT], BF16, tag="zTx", name="zTx")

        def make_attn(side, qs, ks, zTs, fill_cb=None):
            aest = ExitStack()
            s_ps = aest.enter_context(
                tc.tile_pool(name=f"sps{side}", bufs=3, space="PSUM"))
            av_ps = aest.enter_context(
                tc.tile_pool(name=f"avp{side}", bufs=2, space="PSUM"))
            zt_ps = aest.enter_context(
                tc.tile_pool(name=f"ztp{side}", bufs=1, space="PSUM"))
            es_pool = aest.enter_context(tc.tile_pool(name=f"es{side}", bufs=10))
            zt_pool = aest.enter_context(tc.tile_pool(name=f"zt{side}", bufs=6))
            nrm_pool = aest.enter_context(tc.tile_pool(name=f"nr{side}", bufs=12))

            pending = []  # deferred z-transposes: one batch of lookahead so
            # PE gets fresh S work instead of stalling on the normalize chain

            def ztr_flush():
                while pending:
                    zt, col0 = pending.pop(0)
                    for i in range(2):
                        ztp = zt_ps.tile([128, CH * MC], BF16, tag="ztp",
                                         name="ztp")
                        for c in range(CH):
                            nc.tensor.transpose(ztp[:, c * MC:(c + 1) * MC],
                                                zt[i][:, c * 128:(c + 1) * 128],
                                                ident_bf[0:MC, 0:MC])
                        dst3 = zTs.rearrange("p (c t) -> p c t", t=NT)[
                            :, :, col0 + i * MC:col0 + (i + 1) * MC]
                        nc.vector.tensor_copy(
                            dst3, ztp.rearrange("p (c n) -> p c n", n=MC))

            def attn_b(b):
                # emit the previous batch's deferred transposes FIRST so
                # fill_cb-emitted proj units see their zT inputs written
                ztr_flush()
                col0 = b * N
                zt = [zt_pool.tile([MC, DIM], BF16, tag="z", name="z")
                      for _ in range(2)]
                for half in range(2):
                    oaug = [av_ps.tile([MC, 6 * 65], F32, tag="oa", name="oa")
                            for _ in range(2)]
                    for hh in range(6):
                        h = half * 6 + hh
                        c6 = h // 2
                        p0 = (h % 2) * 64
                        q_ap = qs[p0:p0 + 64, c6 * NT + col0:c6 * NT + col0 + N]
                        sps = s_ps.tile([MC, 2 * N], F32, tag="S", name="S")
                        for j in range(2):
                            k_ap = ks[p0:p0 + 64,
                                      c6 * NT + col0 + j * MC:
                                      c6 * NT + col0 + (j + 1) * MC]
                            nc.tensor.matmul(sps[:, j * N:(j + 1) * N],
                                             k_ap, q_ap, start=True, stop=True)
                        expS = es_pool.tile([MC, 2 * N], BF16, tag="expS",
                                            name="expS")
                        with tc.high_priority(offset=150):
                            nc.scalar.activation(expS, sps, AF.Exp, scale=SCALE)
                        for i in range(2):
                            for j in range(2):
                                lhs = expS[:, j * N + i * MC:j * N + (i + 1) * MC]
                                rhs = vt[b][j][:, h * 65:(h + 1) * 65]
                                nc.tensor.matmul(
                                    oaug[i][:, hh * 65:(hh + 1) * 65],
                                    lhs, rhs, start=(j == 0), stop=(j == 1))
                    # normalize frees the oaug psum bank the next half needs;
                    # hoist its scheduler priority over bulk DVE evictions
                    with tc.high_priority(offset=200):
                        for i in range(2):
                            o3 = oaug[i].rearrange("p (h o) -> p h o", o=65)
                            rec = nrm_pool.tile([MC, 6], F32, tag="rec",
                                                name="rec")
                            nc.vector.reciprocal(rec, o3[:, :, 64:65])
                            z3 = zt[i].rearrange(
                                "p (h d) -> p h d",
                                d=64)[:, half * 6:(half + 1) * 6, :]
                            r3 = rec.unsqueeze(2).to_broadcast((MC, 6, 64))
                            nc.vector.tensor_tensor(z3, o3[:, :, 0:64], r3,
                                                    op=ALU.mult)
                    if fill_cb is not None:
                        fill_cb()
                pending.append((zt, col0))

            return aest, attn_b, ztr_flush

        # ------- phases 8-10: one continuous pipeline -------
        # ky and proj units are interleaved into the attention loops: proj
        # unit t only needs the zT batches covering tokens [128t, 128t+128),
        # so it becomes PE filler while ACT grinds the softmax exps.
        fill_est = ExitStack()
        fill_ps = fill_est.enter_context(
            tc.tile_pool(name="fillps", bufs=2, space="PSUM"))

        def proj_unit(zT_s, w_s, bname, od, t, pool=None, split_dma=False):
            stage = stage_pool.tile([128, DIM], BF16, tag="ostg", name="ostg",
                                    bufs=4)
            bt = bias_bc[bname]
            for nf in range(2):
                ps = (pool or fill_ps).tile([128, NF], F32, tag="pp", name="pp")
                for kc in range(CH):
                    nc.tensor.matmul(
                        ps, zT_s[:, kc * NT + t * 128:kc * NT + (t + 1) * 128],
                        w_s[:, kc * DIM + nf * NF:kc * DIM + (nf + 1) * NF],
                        start=(kc == 0), stop=(kc == CH - 1))
                nc.vector.tensor_tensor(stage[:, nf * NF:(nf + 1) * NF], ps,
                                        bt[:, nf * NF:(nf + 1) * NF], op=ALU.add)
                if split_dma:
                    nc.sync.dma_start(
                        out=od[t * 128:(t + 1) * 128, nf * NF:(nf + 1) * NF],
                        in_=stage[:, nf * NF:(nf + 1) * NF])
            if not split_dma:
                nc.sync.dma_start(out=od[t * 128:(t + 1) * 128, :], in_=stage)

        def units_for(zT_s, o1, o2):
            out = []
            for t in range(NROW):
                out.append((zT_s, wp_s, "b1", outs_d[o1], t))
                out.append((zT_s, wp2_s, "b2", outs_d[o2], t))
            return out

        # attn_x loop: ky m-chunks + proj_x units dripped in after each
        # attention half; a reserve of proj_x units is kept for attn_y
        px_units = units_for(zTx, "x1", "xo")
        py_units = units_for(zTy, "y1", "yo")
        RESERVE = 18
        state = {"xi": 0, "yi": 0, "xlim": 0, "ylim": 0}

        def fill_x():
            if state["xi"] < min(state["xlim"], 2 * NROW - RESERVE):
                proj_unit(*px_units[state["xi"]])
                state["xi"] += 1

        def fill_y():
            if state["xi"] < 2 * NROW:
                proj_unit(*px_units[state["xi"]])
                state["xi"] += 1
            elif state["yi"] < state["ylim"]:
                last = state["yi"] >= 2 * NROW - 2
                proj_unit(*py_units[state["yi"]], split_dma=last)
                state["yi"] += 1

        ax_est, attnx_b, attnx_flush = make_attn(0, qx, kx, zTx,
                                                 fill_cb=fill_x)
        for b in range(BC):
            # zTx written through batch b-1 (transposes deferred one batch)
            state["xlim"] = 2 * min(NROW, (N * b) // 128)
            attnx_b(b)
            if b < CH:
                qproj(wk_s, yT_s, ky, b, pool=fill_ps, tag="pp", eng=nc.vector)
        attnx_flush()
        ax_est.close()
        sa_est.close()
        qkx_est.close()
        yT_est.close()
        wk_est.close()

        # attn_y loop: drain the proj_x reserve first, then proj_y as ready
        ticker = [0]

        def fill_y_half():
            ticker[0] += 1
            if ticker[0] % 2 == 0:
                fill_y()

        ay_est, attny_b, attny_flush = make_attn(1, qy, ky, zTy,
                                                 fill_cb=None)
        for b in range(BC):
            state["ylim"] = 2 * min(NROW, (N * b) // 128)
            attny_b(b)
        # only the FINAL flush gates the tail-critical last proj units;
        # hoist just this one over the drain burst
        with tc.high_priority(offset=300):
            attny_flush()
        state["ylim"] = 2 * NROW
        while state["xi"] < 2 * NROW or state["yi"] < 2 * NROW:
            fill_y()
        ay_est.close()
        zTx_est.close()
        zTy_est.close()
        fill_est.close()
        qky_est.close()
        vt_est.close()


def _slab6(a):
    """[768, X] -> [128, 6*X] chunk-major slab: out[p, c*X+x] = a[c*128+p, x]"""
    X = a.shape[1]
    return np.ascontiguousarray(
        a.reshape(CH, 128, X).transpose(1, 0, 2).reshape(128, CH * X))


def _prep_weights(inputs):
    import ml_dtypes
    bf16 = ml_dtypes.bfloat16

    Wqkv = np.asarray(inputs["Wqkv"], np.float32)
    wq = _slab6(Wqkv[:, DIM:2 * DIM]).astype(bf16)
    wk = _slab6(Wqkv[:, 2 * DIM:3 * DIM]).astype(bf16)
    wv = _slab6(Wqkv[:, 3 * DIM:4 * DIM]).astype(bf16)
    wp = np.asarray(inputs["Wproj"], np.float32)
    bp = np.asarray(inputs["bproj"], np.float32).reshape(1, DIM)
    wp64 = wp.astype(np.float64)
    wp2 = (wp64 @ wp64).astype(np.float32)
    bp2 = (bp.astype(np.float64) @ wp64 + bp.astype(np.float64)).astype(np.float32)
    se_w1 = np.asarray(inputs["se_w1"], np.float32)
    sw1m = _slab6(se_w1 / float(N)).astype(bf16)
    sw1x = _slab6(se_w1).astype(bf16)
    sw2 = np.ascontiguousarray(np.asarray(inputs["se_w2"], np.float32)).astype(bf16)
    sa_w = np.asarray(inputs["sa_w"], np.float32)  # [1, 2, 5, 5]
    # cw[(dx,ch), dy] = sa_w[0, ch, dy, dx], mean channel fed as sum -> /DIM
    cw = np.empty((10, 5), np.float32)
    for dx in range(5):
        cw[2 * dx + 0, :] = sa_w[0, 0, :, dx] / float(DIM)
        cw[2 * dx + 1, :] = sa_w[0, 1, :, dx]
    cb = -np.asarray(inputs["sa_b"], np.float32).reshape(1, 1)
    return dict(wq=wq, wk=wk, wv=wv,
                wp=_slab6(wp).astype(bf16), wp2=_slab6(wp2).astype(bf16),
                bp=bp.astype(bf16), bp2=bp2.astype(bf16),
                sw1m=sw1m, sw1x=sw1x, sw2=sw2,
                cw=cw.astype(bf16), cb=cb)


def _in_maps(inputs):
    import ml_dtypes
    bf16 = ml_dtypes.bfloat16
    w = _prep_weights(inputs)
    x = np.asarray(inputs["x"], np.float32).reshape(B, N, DIM)
    y = np.asarray(inputs["y"], np.float32).reshape(B, N, DIM)
    maps = []
    for i in range(NCORES):
        m = dict(w)
        xc = x[i * BC:(i + 1) * BC].reshape(NT, DIM)
        yc = y[i * BC:(i + 1) * BC].reshape(NT, DIM)
        m["xT"] = _slab6(np.ascontiguousarray(xc.T)).astype(bf16)
        m["yT"] = _slab6(np.ascontiguousarray(yc.T)).astype(bf16)
        maps.append(m)
    return maps


def kernel(**inputs):
    from concourse.bass_utils import run_bass_kernel_spmd

    if "nc" not in _COMPILED:
        _COMPILED["nc"] = build_program()
    nc = _COMPILED["nc"]

    res = run_bass_kernel_spmd(nc, _in_maps(inputs), core_ids=list(range(NCORES)))
    outs = []
    for name in ("x1", "y1", "xo", "yo"):
        full = np.concatenate(
            [np.asarray(res.results[i][name], np.float32).reshape(BC, N, DIM)
             for i in range(NCORES)], axis=0)
        outs.append(full)
    return tuple(outs)


# revision 136
# speedup vs baseline: 1.0047x; 1.0047x over previous
"""Trainium2 Bass kernel for nn_Attention_29326036697657 (sparse_attention).

Dual-input attention with SE (channel) / SA (spatial) gates.
Sharding: data-parallel over batch B=64 across 8 cores (8 batches/core).

Algebraic simplifications vs the reference (same as baseline):
  - qxo/qyo/attnx are dead code in the reference -> comp 0 of Wqkv unused.
  - vy = vx (reference quirk) -> only one V, from x's qkv.
  - dots(qx,kx)+dots(qx2,kx) = dots(qx*(1+g), kx)   (SE channel gate)
  - dots(qy,ky)+dots(qy2,ky) = dots(qy*(1+s), ky)   (SA spatial gate scales
    q rows by query position)
  - xo = z @ Wp^2 + (b@Wp + b), computed host-side as wp2/bp2.
Softmax without max-subtraction (logits are O(1)).

v4 design (cost-model driven; 445us -> 224.9us, 1.98x over baseline):
  - All activations/weights bf16 on device (1 cyc/row matmuls); outputs
    written bf16 and widened to f32 on the host.
  - Inputs arrive HOST-pre-transposed as bf16 slabs [128, 6*1152]
    (chunk-major), so no on-device input transposes at all; weights
    host-packed to [128, 6*768] slabs, one DMA each (DMA instruction
    count matters: each holds the shared HWDGE device ~625ns).
  - SE channel gate: stats reduced per qx chunk as it is produced; the
    (1+g) gate is FUSED into the kx psum evictions (dots(q(1+g),k) ==
    dots(q,(1+g)k) for a per-(batch,channel) gate), so no separate
    scaling pass exists on the critical path.
  - SA spatial gate: channel sum/max via gpsimd partition_all_reduce;
    padded grids live in single partitions and are embedded by compute
    engines (no DMA); the 5x5 conv runs as 5 dy-shifted matmuls over an
    x-pre-shifted [10=(dx,ch), b*16*12] operand built with 10 small
    gpsimd DMAs; gate broadcast via gpsimd partition_broadcast.
  - Attention: S/exp/av per (b,head) with the softmax denominator
    produced by a ones-column in the V tiles; z transposed via 6 PE
    transposes into one [128,432] bf16 psum tile + a single DVE
    eviction per (b,i); per-batch transposes deferred one batch.
  - One continuous pipeline: ky and proj units (which only need the zT
    token rows their t-tile covers) are dripped into the attention
    loops after each half-batch, keeping PE busy while ACT grinds the
    192 softmax exps (the second-busiest engine).
  - Bias rows broadcast once via gpsimd partition_broadcast and fused
    into the psum->stage evictions on DVE.
  - ACT runs only {Copy, Exp}: relu is a DVE max0 and every sigmoid is
    computed as 1/(1+exp(-x)) on the Exp table, so the two 1.28us
    mid-kernel ACT table reloads disappear (one load at t~0.7us).
"""

import sys

sys.path.insert(0, "/opt/trn_rl_repo")

from contextlib import ExitStack

import numpy as np

import concourse.bass as bass
import concourse.bacc as bacc_mod
import concourse.bass_isa as bass_isa
import concourse.mybir as mybir
import concourse.tile as tile
from concourse.masks import make_identity

# ---------------------------------------------------------------- constants
DIM = 768
HEADS = 12
PATCH = 12
N = PATCH * PATCH          # 144
B = 64
RED = 16
HID = DIM // RED           # 48
HD = DIM // HEADS          # 64
SCALE = HD ** -0.5         # 0.125

NCORES = 8
BC = B // NCORES           # 8 batches per core
NT = BC * N                # 1152 tokens per core
CH = DIM // 128            # 6 channel chunks
NROW = NT // 128           # 9 row chunks
NF = 384                   # matmul moving-dim chunk
NNF = NT // NF             # 3
MC = 72                    # m/n chunk within one batch (144 = 2*72)

F32 = mybir.dt.float32
F32R = mybir.dt.float32r
BF16 = mybir.dt.bfloat16
AX = mybir.AxisListType
AF = mybir.ActivationFunctionType
ALU = mybir.AluOpType
RO = bass_isa.ReduceOp

_COMPILED = {}


def build_program():
    nc = bacc_mod.Bacc()

    # ---- DRAM I/O (all layouts are host-prepared) ----
    xT_d = nc.dram_tensor("xT", [128, CH * NT], BF16, kind="ExternalInput")
    yT_d = nc.dram_tensor("yT", [128, CH * NT], BF16, kind="ExternalInput")
    wq_d = nc.dram_tensor("wq", [128, CH * DIM], BF16, kind="ExternalInput")
    wk_d = nc.dram_tensor("wk", [128, CH * DIM], BF16, kind="ExternalInput")
    wv_d = nc.dram_tensor("wv", [128, CH * DIM], BF16, kind="ExternalInput")
    wp_d = nc.dram_tensor("wp", [128, CH * DIM], BF16, kind="ExternalInput")
    wp2_d = nc.dram_tensor("wp2", [128, CH * DIM], BF16, kind="ExternalInput")
    bp_d = nc.dram_tensor("bp", [1, DIM], BF16, kind="ExternalInput")
    bp2_d = nc.dram_tensor("bp2", [1, DIM], BF16, kind="ExternalInput")
    sw1m_d = nc.dram_tensor("sw1m", [128, CH * HID], BF16, kind="ExternalInput")
    sw1x_d = nc.dram_tensor("sw1x", [128, CH * HID], BF16, kind="ExternalInput")
    sw2_d = nc.dram_tensor("sw2", [HID, DIM], BF16, kind="ExternalInput")
    cw_d = nc.dram_tensor("cw", [10, 5], BF16, kind="ExternalInput")
    cb_d = nc.dram_tensor("cb", [1, 1], F32, kind="ExternalInput")
    outs_d = {
        nm: nc.dram_tensor(nm, [NT, DIM], BF16, kind="ExternalOutput")
        for nm in ("x1", "y1", "xo", "yo")
    }

    with tile.TileContext(nc) as tc:
        _body(nc, tc, xT_d, yT_d, wq_d, wk_d, wv_d, wp_d, wp2_d, bp_d, bp2_d,
              sw1m_d, sw1x_d, sw2_d, cw_d, cb_d, outs_d)
    nc.compile()
    return nc


def _body(nc, tc, xT_d, yT_d, wq_d, wk_d, wv_d, wp_d, wp2_d, bp_d, bp2_d,
          sw1m_d, sw1x_d, sw2_d, cw_d, cb_d, outs_d):
    est = ExitStack()
    with est:
        # ---------------- const / small tiles ----------------
        const = est.enter_context(tc.tile_pool(name="const", bufs=1))
        ident_bf = const.tile([128, 128], BF16, tag="idbf", name="idbf")
        make_identity(nc, ident_bf)
        cb_sb = const.tile([1, 1], F32, tag="cb", name="cb")
        cw_sb = const.tile([10, 5], BF16, tag="cw", name="cw")
        # touch every activation function once while ACT is idle so Bacc's
        # table-load instructions land at t~0 instead of mid-kernel
        warm = const.tile([1, 1], F32, tag="warm", name="warm")
        nc.vector.memset(warm, 0.5)
        nc.scalar.activation(warm, warm, AF.Exp)

        # ---------------- pools (LIFO nesting per side) ----------------
        # left open order: vt, qky, wk, yT, qkx, sa, wqv, xT, se, vs;
        # closes: se(SE-b), vs/xT/wqv (v done), sa(SA-b), qkx/yT/wk (attn_x
        # done), then pw/stage open and everything lives to the end.
        vt_est = ExitStack()
        vt_pool = vt_est.enter_context(tc.tile_pool(name="vt", bufs=1))
        vt = [[vt_pool.tile([MC, HEADS * 65], BF16, tag=f"v{b}_{j}",
                            name=f"v{b}_{j}") for j in range(2)]
              for b in range(BC)]

        qky_est = ExitStack()
        qky_pool = qky_est.enter_context(tc.tile_pool(name="qky", bufs=1))
        qy = qky_pool.tile([128, CH * NT], BF16, tag="qy", name="qy")
        ky = qky_pool.tile([128, CH * NT], BF16, tag="ky", name="ky")
        wp_s = qky_pool.tile([128, CH * DIM], BF16, tag="wp", name="wp")
        wp2_s = qky_pool.tile([128, CH * DIM], BF16, tag="wp2", name="wp2")
        bias_bc = {}
        bstg = {}
        for bname in ("b1", "b2"):
            bias_bc[bname] = qky_pool.tile([128, DIM], BF16, tag=f"bc{bname}",
                                           name=f"bc{bname}")
            bstg[bname] = qky_pool.tile([1, DIM], BF16, tag=f"bs{bname}",
                                        name=f"bs{bname}")
        stage_pool = qky_pool

        wk_est = ExitStack()
        wk_pool = wk_est.enter_context(tc.tile_pool(name="wkp", bufs=1))
        wk_s = wk_pool.tile([128, CH * DIM], BF16, tag="wk", name="wk")

        yT_est = ExitStack()
        yT_pool = yT_est.enter_context(tc.tile_pool(name="yTp", bufs=1))
        yT_s = yT_pool.tile([128, CH * NT], BF16, tag="yT", name="yT")

        qkx_est = ExitStack()
        qkx_pool = qkx_est.enter_context(tc.tile_pool(name="qkx", bufs=1))
        qx = qkx_pool.tile([128, CH * NT], BF16, tag="qx", name="qx")
        kx = qkx_pool.tile([128, CH * NT], BF16, tag="kx", name="kx")

        sa_est = ExitStack()
        sa_pool = sa_est.enter_context(tc.tile_pool(name="sa", bufs=1))

        wqv_est = ExitStack()
        wqv_pool = wqv_est.enter_context(tc.tile_pool(name="wqv", bufs=1))
        wq_s = wqv_pool.tile([128, CH * DIM], BF16, tag="wq", name="wq")
        wv_s = wqv_pool.tile([128, CH * DIM], BF16, tag="wv", name="wv")

        xT_est = ExitStack()
        xT_pool = xT_est.enter_context(tc.tile_pool(name="xTp", bufs=1))
        xT_s = xT_pool.tile([128, CH * NT], BF16, tag="xT", name="xT")

        # startup DMA order: interleave wq/xT sixths so q matmuls start ASAP
        T6 = CH * DIM // 6
        TT = CH * NT // 6
        for i in range(6):
            nc.sync.dma_start(out=wq_s[:, i * T6:(i + 1) * T6],
                              in_=wq_d[:, i * T6:(i + 1) * T6])
            nc.sync.dma_start(out=xT_s[:, i * TT:(i + 1) * TT],
                              in_=xT_d[:, i * TT:(i + 1) * TT])
        HT = CH * NT // 2
        nc.sync.dma_start(out=yT_s[:, 0:HT], in_=yT_d[:, 0:HT])
        nc.sync.dma_start(out=yT_s[:, HT:], in_=yT_d[:, HT:])
        nc.sync.dma_start(out=wk_s, in_=wk_d[:, :])
        nc.sync.dma_start(out=wv_s, in_=wv_d[:, :])
        nc.sync.dma_start(out=cb_sb, in_=cb_d[:, :])
        nc.sync.dma_start(out=cw_sb, in_=cw_d[:, :])
        nc.sync.dma_start(out=wp_s, in_=wp_d[:, :])
        nc.sync.dma_start(out=wp2_s, in_=wp2_d[:, :])
        for bname, b_d in (("b1", bp_d), ("b2", bp2_d)):
            nc.sync.dma_start(out=bstg[bname], in_=b_d[:, :])
            nc.gpsimd.partition_broadcast(bias_bc[bname], bstg[bname], 128)

        evict_ctr = [0]

        def evict(dst, src):
            # alternate psum->sbuf eviction between DVE and ACT
            if evict_ctr[0] % 2 == 0:
                nc.vector.tensor_copy(dst, src)
            else:
                nc.scalar.copy(dst, src)
            evict_ctr[0] += 1

        # ---------------- phase 1: q projections ----------------
        qkv_est = ExitStack()
        qkv_ps = qkv_est.enter_context(
            tc.tile_pool(name="qkvps", bufs=6, space="PSUM"))

        def qproj(w_s, src_s, dst_s, m, pool=None, tag="qkv", eng=None):
            # one m-chunk of a [768->768] projection, transposed output
            for nf in range(NNF):
                ps = (pool or qkv_ps).tile([128, NF], F32, tag=tag, name=tag)
                for kc in range(CH):
                    nc.tensor.matmul(
                        ps,
                        w_s[:, kc * DIM + m * 128:kc * DIM + (m + 1) * 128],
                        src_s[:, kc * NT + nf * NF:kc * NT + (nf + 1) * NF],
                        start=(kc == 0), stop=(kc == CH - 1))
                dst = dst_s[:, m * NT + nf * NF:m * NT + (nf + 1) * NF]
                if eng is None:
                    evict(dst, ps)
                else:
                    eng.tensor_copy(dst, ps)

        # ------- phase 1+2: qx with SE stats interleaved, then SE fc -------
        se_est = ExitStack()
        se_pool = se_est.enter_context(tc.tile_pool(name="se", bufs=1))
        sw1m_s = se_pool.tile([128, CH * HID], BF16, tag="s1m", name="s1m")
        sw1x_s = se_pool.tile([128, CH * HID], BF16, tag="s1x", name="s1x")
        sw2_s = se_pool.tile([HID, DIM], BF16, tag="sw2", name="sw2")
        nc.sync.dma_start(out=sw1m_s, in_=sw1m_d[:, :])
        nc.sync.dma_start(out=sw1x_s, in_=sw1x_d[:, :])
        nc.sync.dma_start(out=sw2_s, in_=sw2_d[:, :])
        sums = [se_pool.tile([128, BC], BF16, tag=f"sum{c}", name=f"sum{c}")
                for c in range(CH)]
        maxs = [se_pool.tile([128, BC], BF16, tag=f"max{c}", name=f"max{c}")
                for c in range(CH)]
        # SE fc1 accumulates incrementally as each qx chunk's stats land,
        # so g1 (needed by the gate-fused kx evictions) is ready ~when the
        # last qx chunk finishes instead of a full fc-chain latency later.
        se_ps_est = ExitStack()
        se_ps = se_ps_est.enter_context(
            tc.tile_pool(name="seps", bufs=1, space="PSUM"))
        fc1b = se_ps.tile([HID, 2 * BC], F32, tag="fc1", name="fc1")
        fc1 = {0: fc1b[:, 0:BC], 1: fc1b[:, BC:2 * BC]}
        with nc.allow_low_precision(reason="SE gate stats tolerate bf16"):
            for m in range(CH):
                qproj(wq_s, xT_s, qx, m)
                q3 = qx[:, m * NT:(m + 1) * NT].rearrange("p (b n) -> p b n", n=N)
                nc.vector.reduce_sum(sums[m], q3, axis=AX.X)
                nc.vector.reduce_max(maxs[m], q3, axis=AX.X)
                nc.tensor.matmul(fc1[0], sw1m_s[:, m * HID:(m + 1) * HID],
                                 sums[m], start=(m == 0), stop=(m == CH - 1))
                nc.tensor.matmul(fc1[1], sw1x_s[:, m * HID:(m + 1) * HID],
                                 maxs[m], start=(m == 0), stop=(m == CH - 1))

        hidb = se_pool.tile([HID, 2 * BC], BF16, tag="hidb", name="hidb")
        with nc.allow_low_precision(reason="SE gate tolerates bf16"):
            nc.vector.tensor_scalar(hidb, fc1b, 0.0, 0.0, ALU.max, ALU.bypass)
        g1s = []
        lp = nc.allow_low_precision(reason="SE gate tolerates bf16")
        lp.__enter__()
        for c in range(CH):
            ps2 = se_ps.tile([128, 2 * BC], F32, tag="fc2", name="fc2")
            nc.tensor.matmul(ps2[:, 0:BC], sw2_s[:, c * 128:(c + 1) * 128],
                             hidb[:, 0:BC], start=True, stop=True)
            nc.tensor.matmul(ps2[:, BC:2 * BC], sw2_s[:, c * 128:(c + 1) * 128],
                             hidb[:, BC:2 * BC], start=True, stop=True)
            # sigmoid = 1/(1+exp(-x)) using the Exp table (keeps ACT on one
            # activation set -> no mid-kernel table reloads)
            eneg = se_pool.tile([128, 2 * BC], F32, tag="en", name="en", bufs=2)
            nc.scalar.activation(eneg, ps2, AF.Exp, scale=-1.0)
            nc.vector.tensor_scalar(eneg, eneg, 1.0, 0.0, ALU.add, ALU.bypass)
            sg = se_pool.tile([128, 2 * BC], BF16, tag=f"sg{c}", name=f"sg{c}")
            nc.vector.reciprocal(sg, eneg)
            g1 = se_pool.tile([128, BC], BF16, tag=f"g1{c}", name=f"g1{c}")
            nc.vector.tensor_add(g1, sg[:, 0:BC], sg[:, BC:2 * BC])
            g1s.append(g1)
        lp.__exit__(None, None, None)
        se_ps_est.close()

        for m in range(CH):
            qproj(wq_s, yT_s, qy, m)

        # ---------------- phase 3: SA-a (spatial stats of qy) ----------------
        sa_ps_est = ExitStack()
        sa_ps = sa_ps_est.enter_context(
            tc.tile_pool(name="saps", bufs=1, space="PSUM"))
        accm = sa_pool.tile([128, NT], BF16, tag="accm", name="accm")
        accs = sa_pool.tile([128, NT], BF16, tag="accs", name="accs")
        nc.vector.tensor_max(accm, qy[:, 0:NT], qy[:, NT:2 * NT])
        with nc.allow_low_precision(reason="SA mean tolerates bf16"):
            nc.vector.tensor_add(accs, qy[:, 0:NT], qy[:, NT:2 * NT])
            for c in range(2, CH):
                nc.vector.tensor_max(accm, accm, qy[:, c * NT:(c + 1) * NT])
                nc.vector.tensor_add(accs, accs, qy[:, c * NT:(c + 1) * NT])
        pmax = sa_pool.tile([128, NT], BF16, tag="pmax", name="pmax")
        nc.gpsimd.partition_all_reduce(pmax, accm, 128, RO.max)
        pavg = sa_pool.tile([128, NT], BF16, tag="pavg", name="pavg")
        nc.gpsimd.partition_all_reduce(pavg, accs, 128, RO.add)
        # Padded 16x16 grids per channel, each in ONE partition so compute
        # engines can write them (no partition-base-1 access). Channel 0 =
        # mean (as SUM; /DIM folded into conv weight), channel 1 = max.
        mean_pad = sa_pool.tile([1, BC * 256], BF16, tag="mpad", name="mpad")
        max_pad = sa_pool.tile([1, BC * 256], BF16, tag="xpad", name="xpad")
        nc.vector.memset(mean_pad, 0.0)
        nc.vector.memset(max_pad, 0.0)
        mpadw = mean_pad.rearrange("p (b yy xx) -> p b yy xx", yy=16, xx=16)
        xpadw = max_pad.rearrange("p (b yy xx) -> p b yy xx", yy=16, xx=16)
        NG = 2 * N  # 288-col group = 2 batches
        for g in range(4):
            nc.scalar.copy(
                mpadw[0:1, 2 * g:2 * g + 2, 2:14, 2:14],
                pavg[0:1, g * NG:(g + 1) * NG].rearrange(
                    "p (b yy xx) -> p b yy xx", yy=12, xx=12))
            nc.scalar.copy(
                xpadw[0:1, 2 * g:2 * g + 2, 2:14, 2:14],
                pmax[0:1, g * NG:(g + 1) * NG].rearrange(
                    "p (b yy xx) -> p b yy xx", yy=12, xx=12))
        # x-pre-shifted conv operand: opx[(dx,ch), (b, py16, x12)] =
        # grid_ch[b, py, x+dx]; 10 small DMAs, then the 5x5 conv is 5
        # dy-shifted matmuls per 2-batch group contracting over (dx,ch).
        opx = sa_pool.tile([10, BC * 16 * PATCH], BF16, tag="opx", name="opx")
        opx4 = opx.rearrange("p (b yy xx) -> p b yy xx", yy=16, xx=PATCH)
        for dx in range(5):
            for chn, grid in ((0, mpadw), (1, xpadw)):
                nc.gpsimd.dma_start(out=opx4[2 * dx + chn:2 * dx + chn + 1],
                                    in_=grid[:, :, :, dx:dx + PATCH])

        # ---------------- phase 4: kx (SE gate fused into evictions) -------
        # eviction: kx = (1+g) * psum, with g broadcast along n per batch.
        # 288-col tiles (2 batches) so the broadcast view stays b-aligned.
        NK = 2 * N  # 288

        def kproj(m):
            for nf in range(4):
                ps = qkv_ps.tile([128, NK], F32, tag="qkv", name="qkv")
                for kc in range(CH):
                    nc.tensor.matmul(
                        ps,
                        wk_s[:, kc * DIM + m * 128:kc * DIM + (m + 1) * 128],
                        xT_s[:, kc * NT + nf * NK:kc * NT + (nf + 1) * NK],
                        start=(kc == 0), stop=(kc == CH - 1))
                dst3 = kx[:, m * NT + nf * NK:m * NT + (nf + 1) * NK].rearrange(
                    "p (b n) -> p b n", n=N)
                g3 = g1s[m][:, 2 * nf:2 * nf + 2].unsqueeze(2).to_broadcast(
                    (128, 2, N))
                nc.vector.scalar_tensor_tensor(
                    dst3, g3, 1.0, ps.rearrange("p (b n) -> p b n", n=N),
                    ALU.add, ALU.mult)

        for m in range(CH):
            kproj(m)

        # ---------------- phase 6: v (natural layout + resplit) ----------------
        vs_est = ExitStack()
        vs_pool = vs_est.enter_context(tc.tile_pool(name="vs", bufs=4))
        vstage = [None] * NROW
        for t in range(NROW):
            vstage[t] = vs_pool.tile([128, HEADS * 65], BF16, tag="vs",
                                     name="vs")
            ones_ap = vstage[t].rearrange("p (h o) -> p h o", o=65)[:, :, 64:65]
            nc.vector.memset(ones_ap, 1.0)
            for half in range(2):
                ps = qkv_ps.tile([128, NF], F32, tag="qkv", name="qkv")
                for kc in range(CH):
                    nc.tensor.matmul(
                        ps, xT_s[:, kc * NT + t * 128:kc * NT + (t + 1) * 128],
                        wv_s[:, kc * DIM + half * NF:kc * DIM + (half + 1) * NF],
                        start=(kc == 0), stop=(kc == CH - 1))
                dst3 = vstage[t].rearrange("p (h o) -> p h o", o=65)[
                    :, half * 6:(half + 1) * 6, 0:64]
                evict(dst3, ps.rearrange("p (h d) -> p h d", d=64))
            # re-split each (b,j) slab to its vt tile as soon as covered
            for b in range(BC):
                for j in range(2):
                    row0 = b * N + j * MC
                    if (row0 + MC - 1) // 128 != t:
                        continue
                    pos = 0
                    while pos < MC:
                        tt = (row0 + pos) // 128
                        r0 = (row0 + pos) % 128
                        cnt = min(128 - r0, MC - pos)
                        nc.sync.dma_start(out=vt[b][j][pos:pos + cnt, :],
                                          in_=vstage[tt][r0:r0 + cnt, :])
                        pos += cnt

        vs_est.close()
        se_est.close()
        xT_est.close()
        wqv_est.close()

        # ---------------- phase 7: SA-b (conv gate, scale qy) ----------------
        # 1+sigmoid(x+cb) computed as 1 + 1/(1+exp(-x-cb)); the ACT op uses
        # the Exp table (cb arrives host-negated in cb_sb)
        t_row1 = sa_pool.tile([1, NT], BF16, tag="trow1", name="trow1")
        for g in range(4):
            ps = sa_ps.tile([1, NG], F32, tag="sam", name="sam")
            for dy in range(5):
                v = opx4[:, 2 * g:2 * g + 2, dy:dy + PATCH, :]
                nc.tensor.matmul(ps, cw_sb[:, dy:dy + 1],
                                 v.rearrange("p b yy xx -> p b (yy xx)"),
                                 start=(dy == 0), stop=(dy == 4))
            erow = sa_pool.tile([1, NG], F32, tag="er", name="er", bufs=2)
            nc.scalar.activation(erow, ps, AF.Exp, scale=-1.0, bias=cb_sb)
            nc.vector.tensor_scalar(erow, erow, 1.0, 0.0, ALU.add, ALU.bypass)
            rrow = sa_pool.tile([1, NG], F32, tag="rr", name="rr", bufs=2)
            nc.vector.reciprocal(rrow, erow)
            with nc.allow_low_precision(reason="SA gate tolerates bf16"):
                nc.vector.tensor_scalar(t_row1[:, g * NG:(g + 1) * NG], rrow,
                                        1.0, 0.0, ALU.add, ALU.bypass)
        t_bc = sa_pool.tile([128, NT], BF16, tag="tbc", name="tbc")
        nc.gpsimd.partition_broadcast(t_bc, t_row1, 128)
        with tc.high_priority(offset=-1500):
            for c in range(CH):
                qslice = qy[:, c * NT:(c + 1) * NT]
                nc.vector.tensor_tensor(qslice, qslice, t_bc, op=ALU.mult)
        sa_ps_est.close()
        qkv_est.close()

        # ---------------- attention ----------------
        # right-side stack: zTy under zTx (zTx closes first, after proj_x)
        zTy_est = ExitStack()
        zTy_pool = zTy_est.enter_context(
            tc.tile_pool(name="zTy", bufs=1, side="right"))
        zTy = zTy_pool.tile([128, CH * NT], BF16, tag="zTy", name="zTy")
        zTx_est = ExitStack()
        zTx_pool = zTx_est.enter_context(
            tc.tile_pool(name="zTx", bufs=1, side="right"))
        zTx = zTx_pool.tile([128, CH * NT], BF16, tag="zTx", name="zTx")

        def make_attn(side, qs, ks, zTs, fill_cb=None):
            aest = ExitStack()
            s_ps = aest.enter_context(
                tc.tile_pool(name=f"sps{side}", bufs=3, space="PSUM"))
            av_ps = aest.enter_context(
                tc.tile_pool(name=f"avp{side}", bufs=2, space="PSUM"))
            zt_ps = aest.enter_context(
                tc.tile_pool(name=f"ztp{side}", bufs=1, space="PSUM"))
            es_pool = aest.enter_context(tc.tile_pool(name=f"es{side}", bufs=10))
            zt_pool = aest.enter_context(tc.tile_pool(name=f"zt{side}", bufs=6))
            nrm_pool = aest.enter_context(tc.tile_pool(name=f"nr{side}", bufs=12))

            pending = []  # deferred z-transposes: one batch of lookahead so
            # PE gets fresh S work instead of stalling on the normalize chain

            def ztr_flush():
                while pending:
                    zt, col0 = pending.pop(0)
                    for i in range(2):
                        ztp = zt_ps.tile([128, CH * MC], BF16, tag="ztp",
                                         name="ztp")
                        for c in range(CH):
                            nc.tensor.transpose(ztp[:, c * MC:(c + 1) * MC],
                                                zt[i][:, c * 128:(c + 1) * 128],
                                                ident_bf[0:MC, 0:MC])
                        dst3 = zTs.rearrange("p (c t) -> p c t", t=NT)[
                            :, :, col0 + i * MC:col0 + (i + 1) * MC]
                        nc.vector.tensor_copy(
                            dst3, ztp.rearrange("p (c n) -> p c n", n=MC))

            def attn_b(b):
                # emit the previous batch's deferred transposes FIRST so
                # fill_cb-emitted proj units see their zT inputs written
                ztr_flush()
                col0 = b * N
                zt = [zt_pool.tile([MC, DIM], BF16, tag="z", name="z")
                      for _ in range(2)]
                for half in range(2):
                    oaug = [av_ps.tile([MC, 6 * 65], F32, tag="oa", name="oa")
                            for _ in range(2)]
                    for hh in range(6):
                        h = half * 6 + hh
                        c6 = h // 2
                        p0 = (h % 2) * 64
                        q_ap = qs[p0:p0 + 64, c6 * NT + col0:c6 * NT + col0 + N]
                        sps = s_ps.tile([MC, 2 * N], F32, tag="S", name="S")
                        for j in range(2):
                            k_ap = ks[p0:p0 + 64,
                                      c6 * NT + col0 + j * MC:
                                      c6 * NT + col0 + (j + 1) * MC]
                            nc.tensor.matmul(sps[:, j * N:(j + 1) * N],
                                             k_ap, q_ap, start=True, stop=True)
                        expS = es_pool.tile([MC, 2 * N], BF16, tag="expS",
                                            name="expS")
                        with tc.high_priority(offset=150):
                            nc.scalar.activation(expS, sps, AF.Exp, scale=SCALE)
                        for i in range(2):
                            for j in range(2):
                                lhs = expS[:, j * N + i * MC:j * N + (i + 1) * MC]
                                rhs = vt[b][j][:, h * 65:(h + 1) * 65]
                                nc.tensor.matmul(
                                    oaug[i][:, hh * 65:(hh + 1) * 65],
                                    lhs, rhs, start=(j == 0), stop=(j == 1))
                    # normalize frees the oaug psum bank the next half needs;
                    # hoist its scheduler priority over bulk DVE evictions
                    with tc.high_priority(offset=200):
                        for i in range(2):
                            o3 = oaug[i].rearrange("p (h o) -> p h o", o=65)
                            rec = nrm_pool.tile([MC, 6], F32, tag="rec",
                                                name="rec")
                            nc.vector.reciprocal(rec, o3[:, :, 64:65])
                            z3 = zt[i].rearrange(
                                "p (h d) -> p h d",
                                d=64)[:, half * 6:(half + 1) * 6, :]
                            r3 = rec.unsqueeze(2).to_broadcast((MC, 6, 64))
                            nc.vector.tensor_tensor(z3, o3[:, :, 0:64], r3,
                                                    op=ALU.mult)
                    if fill_cb is not None:
                        fill_cb()
                pending.append((zt, col0))

            return aest, attn_b, ztr_flush

        # ------- phases 8-10: one continuous pipeline -------
        # ky and proj units are interleaved into the attention loops: proj
        # unit t only needs the zT batches covering tokens [128t, 128t+128),
        # so it becomes PE filler while ACT grinds the softmax exps.
        fill_est = ExitStack()
        fill_ps = fill_est.enter_context(
            tc.tile_pool(name="fillps", bufs=2, space="PSUM"))

        def proj_unit(zT_s, w_s, bname, od, t, pool=None, split_dma=False):
            stage = stage_pool.tile([128, DIM], BF16, tag="ostg", name="ostg",
                                    bufs=4)
            bt = bias_bc[bname]
            for nf in range(2):
                ps = (pool or fill_ps).tile([128, NF], F32, tag="pp", name="pp")
                for kc in range(CH):
                    nc.tensor.matmul(
                        ps, zT_s[:, kc * NT + t * 128:kc * NT + (t + 1) * 128],
                        w_s[:, kc * DIM + nf * NF:kc * DIM + (nf + 1) * NF],
                        start=(kc == 0), stop=(kc == CH - 1))
                nc.vector.tensor_tensor(stage[:, nf * NF:(nf + 1) * NF], ps,
                                        bt[:, nf * NF:(nf + 1) * NF], op=ALU.add)
                if split_dma:
                    nc.sync.dma_start(
                        out=od[t * 128:(t + 1) * 128, nf * NF:(nf + 1) * NF],
                        in_=stage[:, nf * NF:(nf + 1) * NF])
            if not split_dma:
                nc.sync.dma_start(out=od[t * 128:(t + 1) * 128, :], in_=stage)

        def units_for(zT_s, o1, o2):
            out = []
            for t in range(NROW):
                out.append((zT_s, wp_s, "b1", outs_d[o1], t))
                out.append((zT_s, wp2_s, "b2", outs_d[o2], t))
            return out

        # attn_x loop: ky m-chunks + proj_x units dripped in after each
        # attention half; a reserve of proj_x units is kept for attn_y
        px_units = units_for(zTx, "x1", "xo")
        py_units = units_for(zTy, "y1", "yo")
        RESERVE = 18
        state = {"xi": 0, "yi": 0, "xlim": 0, "ylim": 0}

        def fill_x():
            if state["xi"] < min(state["xlim"], 2 * NROW - RESERVE):
                proj_unit(*px_units[state["xi"]])
                state["xi"] += 1

        def fill_y():
            if state["xi"] < 2 * NROW:
                proj_unit(*px_units[state["xi"]])
                state["xi"] += 1
            elif state["yi"] < state["ylim"]:
                last = state["yi"] >= 2 * NROW - 2
                proj_unit(*py_units[state["yi"]], split_dma=last)
                state["yi"] += 1

        ax_est, attnx_b, attnx_flush = make_attn(0, qx, kx, zTx,
                                                 fill_cb=fill_x)
        for b in range(BC):
            # zTx written through batch b-1 (transposes deferred one batch)
            state["xlim"] = 2 * min(NROW, (N * b) // 128)
            attnx_b(b)
            if b < CH:
                qproj(wk_s, yT_s, ky, b, pool=fill_ps, tag="pp", eng=nc.vector)
        attnx_flush()
        ax_est.close()
        sa_est.close()
        qkx_est.close()
        yT_est.close()
        wk_est.close()

        # attn_y loop: drain the proj_x reserve first, then proj_y as ready
        ticker = [0]

        def fill_y_half():
            ticker[0] += 1
            if ticker[0] % 2 == 0:
                fill_y()

        ay_est, attny_b, attny_flush = make_attn(1, qy, ky, zTy,
                                                 fill_cb=None)
        for b in range(BC):
            state["ylim"] = 2 * min(NROW, (N * b) // 128)
            attny_b(b)
        # only the FINAL flush gates the tail-critical last proj units;
        # hoist just this one over the drain burst
        with tc.high_priority(offset=300):
            attny_flush()
        state["ylim"] = 2 * NROW
        while state["xi"] < 2 * NROW or state["yi"] < 2 * NROW:
            fill_y()
        ay_est.close()
        zTx_est.close()
        zTy_est.close()
        fill_est.close()
        qky_est.close()
        vt_est.close()


def _slab6(a):
    """[768, X] -> [128, 6*X] chunk-major slab: out[p, c*X+x] = a[c*128+p, x]"""
    X = a.shape[1]
    return np.ascontiguousarray(
        a.reshape(CH, 128, X).transpose(1, 0, 2).reshape(128, CH * X))


def _prep_weights(inputs):
    import ml_dtypes
    bf16 = ml_dtypes.bfloat16

    Wqkv = np.asarray(inputs["Wqkv"], np.float32)
    wq = _slab6(Wqkv[:, DIM:2 * DIM]).astype(bf16)
    wk = _slab6(Wqkv[:, 2 * DIM:3 * DIM]).astype(bf16)
    wv = _slab6(Wqkv[:, 3 * DIM:4 * DIM]).astype(bf16)
    wp = np.asarray(inputs["Wproj"], np.float32)
    bp = np.asarray(inputs["bproj"], np.float32).reshape(1, DIM)
    wp64 = wp.astype(np.float64)
    wp2 = (wp64 @ wp64).astype(np.float32)
    bp2 = (bp.astype(np.float64) @ wp64 + bp.astype(np.float64)).astype(np.float32)
    se_w1 = np.asarray(inputs["se_w1"], np.float32)
    sw1m = _slab6(se_w1 / float(N)).astype(bf16)
    sw1x = _slab6(se_w1).astype(bf16)
    sw2 = np.ascontiguousarray(np.asarray(inputs["se_w2"], np.float32)).astype(bf16)
    sa_w = np.asarray(inputs["sa_w"], np.float32)  # [1, 2, 5, 5]
    # cw[(dx,ch), dy] = sa_w[0, ch, dy, dx], mean channel fed as sum -> /DIM
    cw = np.empty((10, 5), np.float32)
    for dx in range(5):
        cw[2 * dx + 0, :] = sa_w[0, 0, :, dx] / float(DIM)
        cw[2 * dx + 1, :] = sa_w[0, 1, :, dx]
    cb = -np.asarray(inputs["sa_b"], np.float32).reshape(1, 1)
    return dict(wq=wq, wk=wk, wv=wv,
                wp=_slab6(wp).astype(bf16), wp2=_slab6(wp2).astype(bf16),
                bp=bp.astype(bf16), bp2=bp2.astype(bf16),
                sw1m=sw1m, sw1x=sw1x, sw2=sw2,
                cw=cw.astype(bf16), cb=cb)


def _in_maps(inputs):
    import ml_dtypes
    bf16 = ml_dtypes.bfloat16
    w = _prep_weights(inputs)
    x = np.asarray(inputs["x"], np.float32).reshape(B, N, DIM)
    y = np.asarray(inputs["y"], np.float32).reshape(B, N, DIM)
    maps = []
    for i in range(NCORES):
        m = dict(w)
        xc = x[i * BC:(i + 1) * BC].reshape(NT, DIM)
        yc = y[i * BC:(i + 1) * BC].reshape(NT, DIM)
        m["xT"] = _slab6(np.ascontiguousarray(xc.T)).astype(bf16)
        m["yT"] = _slab6(np.ascontiguousarray(yc.T)).astype(bf16)
        maps.append(m)
    return maps


def kernel(**inputs):
    from concourse.bass_utils import run_bass_kernel_spmd

    if "nc" not in _COMPILED:
        _COMPILED["nc"] = build_program()
    nc = _COMPILED["nc"]

    res = run_bass_kernel_spmd(nc, _in_maps(inputs), core_ids=list(range(NCORES)))
    outs = []
    for name in ("x1", "y1", "xo", "yo"):
        full = np.concatenate(
            [np.asarray(res.results[i][name], np.float32).reshape(BC, N, DIM)
             for i in range(NCORES)], axis=0)
        outs.append(full)
    return tuple(outs)
